# revision 3
# baseline (speedup 1.0000x reference)
"""Trainium2 Bass kernel for per-head bilinear graph attention.

Reference computation (B=4, N=2048, IN=256, H=8, ATN=32):
    xt     = einsum('bni,hio->bhno', x, W) + b          # [B,H,N,32]
    xC     = einsum('bhno,hpo->bhnp', xt, C)            # [B,H,N,32]
    scores = einsum('bhnp,bhmp->bhnm', xC, xt)          # [B,H,N,N]
    alpha  = tanh(scores * adj[:,None])                 # [B,H,N,N]
    heads  = einsum('bhnm,bhmo->bhno', alpha, xt)       # [B,H,N,32]
    out    = concat heads on feature dim                # [B,N,256]

Sharding: 8 cores = 4 batches x 2 head-groups (4 heads each). Fully
data-parallel, no collectives. Each core computes out[b, :, hg*128:(hg+1)*128]
transposed ([128, 2048]); the host transposes back and concatenates.

Device-side layout is fully transposed ("T" = [feature/m, n]):
    xtT  [128(4h x 32o), 2048n]   stacked per-head xt^T (bias included)
    xCT  [128(4h x 32p), 2048n]   stacked per-head xC^T
    sT   [128m, n]     = scores[n, m]   (psum, per m-chunk per head)
    z    = sT * adjT   (adjT host-pretransposed so it is [m, n])
    alphaT = tanh(z)
    outT [128(4h x 32o), 2048n] accumulated in psum over 16 m-chunks

Engine budget per core: the kernel sits at the DVE+ACT joint capacity
floor (TRN2 matmul psum output is fp32-only, so every score element must
exit PSUM through a 1x fp32 DVE/ACT op; tanh is ACT-only at 0.83ns/col).
The LP optimum routes ~12-14 of the 128 multiply units through an ACT
cast + GPSIMD multiply and fuses the rest with the psum exit on DVE:
ACT ~= DVE ~= 135us busy. Everything else is scheduling:
 - per-mc z/alpha tiles [P,4,1024] with the tanh pipelined ONE m-chunk
   behind the multiplies (tanh(mc-1) emitted after mc's scores+mults),
   so the GPSIMD cast fires a full tanh+2 mults (~6us) before its
   consumer and ACT always has a banked z buffer to chew on.
 - no zero-seed matmuls: has_written clears are per-partition-slice on
   this HW (the baseline's diagonal xCT matmuls with start=True already
   relied on that), so heads/proj groups use start=(first k-chunk).
 - prologue: xT chunk0 DMA issued FIRST (it heads the critical chain),
   then W, bias; xT1-3 + ident on the gpsimd SWDGE queue. Only proj/xC
   chunks 0,1 gate the start; chunks 2,3 + xt4 transpose groups are
   spread one-per-mc through nh0's early m-chunks.
 - tail: mc15's tanh+heads+output copy are emitted per-q so the last
   512-col output DMA chases the last matmul by <1us.

Key facts baked into this design (from HW traces + the CoreSim cost
model source):
 - DVE tensor_tensor 2x mode needs ALL operands 2-byte; fp32 psum scores
   therefore pin the mask-multiply at 1x. TRN2 matmul cannot write bf16
   psum (is_transpose=True really is a different PE datapath - measured
   garbage), and DMA has no PSUM port, so there is no cheaper exit.
 - GPSIMD cannot access PSUM; its multiplies need an ACT cast first
   (1.0us/unit ACT buys 1.15us/unit off DVE - profitable only while
   ACT's tanh load leaves it slack).
 - K=32 scores matmuls are packed pairwise into PE row-groups
   (tile_position) with [128,2,512] psum tiles; outT uses 4-way
   col-group packing. PSUM: 3x2-bank scores slots + 2x1-bank output
   accumulators = 8 banks.
"""

import sys
import types

import numpy as np
import ml_dtypes

BF16_NP = ml_dtypes.bfloat16


def _ensure_axon_ntff_hook():
    """Provide antenv.axon_hooks if the image lacks it, so
    run_bass_kernel_spmd(trace=True) can capture NTFF profiles instead of
    crashing on the import. No-op when the real module exists."""
    try:
        import antenv.axon_hooks  # noqa: F401

        return
    except ImportError:
        pass
    mod = types.ModuleType("antenv.axon_hooks")
    _state = {"hook": None}
    mod.set_axon_ntff_profile_hook = lambda h: _state.__setitem__("hook", h)
    mod.get_axon_ntff_profile_hook = lambda: _state["hook"]
    sys.modules["antenv.axon_hooks"] = mod
    try:
        import antenv

        antenv.axon_hooks = mod
    except ImportError:
        pass
    try:
        from trn_agent_boot.trn_boot import _ntff_profile_via_ctypes

        mod.set_axon_ntff_profile_hook(
            _ntff_profile_via_ctypes("/opt/axon/libaxon_pjrt.so")
        )
    except Exception:
        pass


_ensure_axon_ntff_hook()

from concourse import bacc, mybir, tile
import concourse.bass as bass
from concourse.bass_utils import run_bass_kernel_spmd

F32 = mybir.dt.float32
BF16 = mybir.dt.bfloat16
AF = mybir.ActivationFunctionType
ALU = mybir.AluOpType

P = 128
B, N, IN_DIM, H, ATN = 4, 2048, 256, 8, 32
NH = 4                # heads per core
NCORES = 8
MC = N // P           # 16 m-chunks
IC = IN_DIM // P      # 2 contraction chunks for the input projection

# m-chunks whose (hp=0, q=0) multiply slice is routed ACT-cast -> GPSIMD
# (per n-half). The cast is emitted before tanh(mc-1), giving the Pool
# multiply ~6us of latency margin before tanh(mc) needs its z slice.
POOL_MCS = (2, 4, 6, 8, 10, 12, 14)

_CACHE = {}


def build_graph():
    nc = bacc.Bacc("TRN2", target_bir_lowering=False, debug=False)

    xT_d = nc.dram_tensor("xT", [IN_DIM, N], BF16, kind="ExternalInput")
    id_d = nc.dram_tensor("ident", [P, P], BF16, kind="ExternalInput")
    adjT_d = nc.dram_tensor("adjT", [N, N], BF16, kind="ExternalInput")
    # weights: [P, IC*NH*ATN] W-part ++ [P, ATN] C^T-part, one fast DMA
    W_d = nc.dram_tensor("Wt", [P, IC * NH * ATN + ATN], BF16, kind="ExternalInput")
    b_d = nc.dram_tensor("bias", [P, 1], F32, kind="ExternalInput")
    out_d = nc.dram_tensor("out", [P, N], BF16, kind="ExternalOutput")

    with tile.TileContext(nc) as tc:
        with (
            tc.tile_pool(name="const", bufs=1) as cp,
            tc.tile_pool(name="adj", bufs=6) as adjp,
            tc.tile_pool(name="z", bufs=5) as zp,
            tc.tile_pool(name="alpha", bufs=4) as alp,
            tc.tile_pool(name="cast", bufs=3) as scp,
            tc.tile_pool(name="ps_o", bufs=2, space="PSUM") as ps_o,
            tc.tile_pool(name="ps_s", bufs=3, space="PSUM") as ps_s,
        ):
            # xT chunk0 heads the serial critical chain (proj -> xC ->
            # scores -> mult -> tanh): issue it FIRST on the sync queue.
            # DMA issues serialize at ~0.65us each per sequencer.
            xT_sb = cp.tile([P, IC, N], BF16)
            xT_src = xT_d[:].rearrange("(c p) n -> p c n", p=P)
            nc.sync.dma_start(
                xT_sb[:, :, bass.ts(0, 512)], xT_src[:, :, bass.ts(0, 512)]
            )
            Wall_sb = cp.tile([P, IC * NH * ATN + ATN], BF16)
            nc.sync.dma_start(Wall_sb[:], W_d[:])
            b_sb = cp.tile([P, 1], F32)
            nc.sync.dma_start(b_sb[:], b_d[:])
            for nq in range(1, N // 512):
                nc.gpsimd.dma_start(
                    xT_sb[:, :, bass.ts(nq, 512)],
                    xT_src[:, :, bass.ts(nq, 512)],
                )
            ident = cp.tile([P, P], BF16)
            nc.gpsimd.dma_start(ident[:], id_d[:])
            W_sb = Wall_sb[:, : IC * NH * ATN].rearrange(
                "p (c h o) -> p c h o", c=IC, h=NH
            )
            CT_sb = Wall_sb[:, IC * NH * ATN :]

            xtT = cp.tile([P, N], BF16)
            xCT = cp.tile([P, N], BF16)
            xt4 = cp.tile([P, MC, P], BF16)
            out_sb = cp.tile([P, N], BF16)

            # --- prologue pieces ---
            def emit_xtT(nq):
                # xtT[32h+o, n] = sum_i W[h,i,o] x[n,i] + b[h,o].
                # c-outer/h-inner so the 4 col-groups run concurrently in
                # the PE array; start=True on the first c-chunk per group
                # (has_written clear is per-partition-slice). The bias
                # rides on the ACT copy out of psum.
                pt = ps_s.tile([P, 1024], F32, tag="s", name=f"pj_{nq}")
                for c in range(IC):
                    for h in range(NH):
                        nc.tensor.matmul(
                            pt[bass.ts(h, ATN), :512],
                            W_sb[:, c, h, :],
                            xT_sb[:, c, bass.ts(nq, 512)],
                            start=(c == 0),
                            stop=(c == IC - 1),
                            tile_position=(0, h * ATN),
                            skip_group_check=True,
                        )
                nc.scalar.activation(
                    xtT[:, bass.ts(nq, 512)], pt[:, :512], AF.Identity, bias=b_sb[:]
                )

            def emit_xCT(nq, on_act=True):
                # xCT[32h+p, n] = sum_o C[h,p,o] xt[n,o]; diagonal 32x32
                # tiles run concurrently in distinct row+col groups.
                pt = ps_s.tile([P, 1024], F32, tag="s", name=f"xc_{nq}")
                for h in range(NH):
                    nc.tensor.matmul(
                        pt[bass.ts(h, ATN), :512],
                        CT_sb[bass.ts(h, ATN), :],
                        xtT[bass.ts(h, ATN), bass.ts(nq, 512)],
                        start=True,
                        stop=True,
                        tile_position=(h * ATN, h * ATN),
                        skip_group_check=True,
                    )
                if on_act:
                    nc.scalar.copy(xCT[:, bass.ts(nq, 512)], pt[:, :512])
                else:
                    nc.vector.tensor_copy(xCT[:, bass.ts(nq, 512)], pt[:, :512])

            def emit_xt4(g):
                # xt4[m_local, mc, f] = xt[mc*128+m_local, f]: PE transposes
                # of xtT, 4 m-chunks per psum tile (cycled through a ps_s
                # slot). Copies on DVE (bf16 psum -> bf16 sbuf, 2x path).
                pt = ps_s.tile([P, 4, P], BF16, tag="s", name=f"tr_{g}")
                for k in range(4):
                    nc.tensor.transpose(
                        pt[:, k, :], xtT[:, bass.ts(4 * g + k, P)], ident[:]
                    )
                nc.vector.tensor_copy(xt4[:, bass.ds(4 * g, 4), :], pt[:])

            # Only projection chunks 0,1 (and transpose group 0) gate the
            # first m-chunk; chunks 2,3 and the later transpose groups are
            # emitted inside the main loop where PE has slack.
            for nq in range(2):
                emit_xtT(nq)
                emit_xCT(nq)
            emit_xt4(0)
            PROLOG_AT = {1: lambda: emit_xtT(2),
                         2: lambda: emit_xCT(2, on_act=False),
                         3: lambda: emit_xt4(1),
                         4: lambda: emit_xtT(3),
                         5: lambda: emit_xCT(3, on_act=False),
                         6: lambda: emit_xt4(2),
                         7: lambda: emit_xt4(3)}

            # --- main loop: n-half outer, m-chunks inner, tanh lagging the
            # multiplies by one m-chunk ---
            NHALF = N // 1024
            for nh in range(NHALF):
                # Two 1-bank accumulators; the q0 output copy fires as soon
                # as q0's last matmul stops, overlapping q1's.
                po_q = [
                    ps_o.tile([P, 512], F32, tag="po", name=f"po_{nh}_{q}")
                    for q in range(2)
                ]

                def emit_heads(mc, alpha, q, nh=nh):
                    for h in range(NH):
                        nc.tensor.matmul(
                            po_q[q][bass.ts(h, ATN), :],
                            xt4[:, mc, bass.ts(h, ATN)],
                            alpha[:, h, bass.ts(q, 512)],
                            start=(mc == 0),
                            stop=(mc == MC - 1),
                            tile_position=(0, h * ATN),
                            skip_group_check=True,
                        )

                def finish(mc, nh=nh):
                    # tanh + heads for m-chunk mc (runs one mc behind the
                    # multiplies so ACT always has banked z to consume).
                    alpha = alp.tile([P, NH, 1024], BF16, tag="alpha",
                                     name=f"al_{nh}_{mc}")
                    if mc < MC - 1:
                        nc.scalar.activation(alpha[:], zbuf[mc % 2][:], AF.Tanh)
                        for q in range(2):
                            emit_heads(mc, alpha, q)
                    else:
                        # tail: per-q tanh so the last output DMA chases
                        # the last matmul closely
                        for q in range(2):
                            nc.scalar.activation(
                                alpha[:, :, bass.ts(q, 512)],
                                zbuf[mc % 2][:, :, bass.ts(q, 512)],
                                AF.Tanh,
                            )
                            emit_heads(mc, alpha, q)
                            nc.vector.tensor_copy(
                                out_sb[:, bass.ds(nh * 1024 + q * 512, 512)],
                                po_q[q][:],
                            )
                            nc.sync.dma_start(
                                out_d[:, bass.ds(nh * 1024 + q * 512, 512)],
                                out_sb[:, bass.ds(nh * 1024 + q * 512, 512)],
                            )

                zbuf = [None, None]
                for mc in range(MC):
                    adjt = adjp.tile([P, 1024], BF16, tag="adj")
                    nc.sync.dma_start(
                        adjt[:], adjT_d[bass.ts(mc, P), bass.ds(nh * 1024, 1024)]
                    )
                    zb = zp.tile([P, NH, 1024], BF16, tag="z",
                                 name=f"zb_{nh}_{mc}")
                    zbuf[mc % 2] = zb
                    pool_mc = mc in POOL_MCS
                    for q in range(2):
                        for hp in range(NH // 2):
                            s2 = ps_s.tile([P, 2, 512], F32, tag="s")
                            for j in range(2):
                                h = 2 * hp + j
                                nc.tensor.matmul(
                                    s2[:, j, :],
                                    xtT[bass.ts(h, ATN), bass.ts(mc, P)],
                                    xCT[
                                        bass.ts(h, ATN),
                                        bass.ds(nh * 1024 + q * 512, 512),
                                    ],
                                    start=True,
                                    stop=True,
                                    tile_position=(h * ATN, 0),
                                    skip_group_check=True,
                                )
                            zsl = zb[:, bass.ds(2 * hp, 2), bass.ts(q, 512)]
                            adj_b = adjt[:, None, bass.ts(q, 512)].to_broadcast(
                                (P, 2, 512)
                            )
                            if pool_mc and q == 0 and hp == 0:
                                # ACT casts psum->bf16; the otherwise-idle
                                # GPSIMD engine does the multiply (it has
                                # no PSUM port, hence the cast).
                                sc = scp.tile([P, 2, 512], BF16, tag="cast")
                                nc.scalar.copy(sc[:], s2[:])
                                nc.gpsimd.tensor_tensor(zsl, sc[:], adj_b, ALU.mult)
                            else:
                                nc.vector.tensor_tensor(zsl, s2[:], adj_b, ALU.mult)
                    if nh == 0 and mc in PROLOG_AT:
                        PROLOG_AT[mc]()
                    if mc >= 1:
                        finish(mc - 1)
                finish(MC - 1)

    nc.compile()
    return nc


def _get_graph():
    if "nc" not in _CACHE:
        _CACHE["nc"] = build_graph()
    return _CACHE["nc"]


def make_in_maps(x, adj, W, b, C):
    in_maps = []
    for core in range(NCORES):
        bb = core // 2
        hg = core % 2
        hs = slice(hg * NH, (hg + 1) * NH)
        Wt = (
            W[hs]
            .reshape(NH, IC, P, ATN)
            .transpose(2, 1, 0, 3)
            .reshape(P, IC * NH * ATN)
        )
        CTt = C[hs].transpose(0, 2, 1).reshape(NH * ATN, ATN)
        in_maps.append(
            {
                "xT": np.ascontiguousarray(x[bb].T).astype(BF16_NP),
                "ident": np.eye(P, dtype=np.float32).astype(BF16_NP),
                "adjT": np.ascontiguousarray(adj[bb].T).astype(BF16_NP),
                "Wt": np.ascontiguousarray(
                    np.concatenate([Wt, CTt], axis=1)
                ).astype(BF16_NP),
                "bias": np.ascontiguousarray(b[hs].reshape(P, 1)),
            }
        )
    return in_maps


LAST_RESULT = None


def kernel(x, adj, W, b, C):
    global LAST_RESULT
    x = np.asarray(x, dtype=np.float32)
    adj = np.asarray(adj, dtype=np.float32)
    W = np.asarray(W, dtype=np.float32)
    b = np.asarray(b, dtype=np.float32)
    C = np.asarray(C, dtype=np.float32)

    nc = _get_graph()
    in_maps = make_in_maps(x, adj, W, b, C)
    # First execution warms the device clocks (p-states ramp under load);
    # the second, profiled execution then runs at the steady-state clock.
    run_bass_kernel_spmd(nc, in_maps, core_ids=list(range(NCORES)))
    res = run_bass_kernel_spmd(nc, in_maps, core_ids=list(range(NCORES)))
    LAST_RESULT = res

    out = np.empty((B, N, H * ATN), dtype=np.float32)
    for core in range(NCORES):
        bb = core // 2
        hg = core % 2
        out[bb, :, hg * P : (hg + 1) * P] = (
            res.results[core]["out"].astype(np.float32).T
        )
    return out


# revision 6
# speedup vs baseline: 1.1907x; 1.1907x over previous
"""Trainium2 Bass kernel for per-head bilinear graph attention.

Reference computation (B=4, N=2048, IN=256, H=8, ATN=32):
    xt     = einsum('bni,hio->bhno', x, W) + b          # [B,H,N,32]
    xC     = einsum('bhno,hpo->bhnp', xt, C)            # [B,H,N,32]
    scores = einsum('bhnp,bhmp->bhnm', xC, xt)          # [B,H,N,N]
    alpha  = tanh(scores * adj[:,None])                 # [B,H,N,N]
    heads  = einsum('bhnm,bhmo->bhno', alpha, xt)       # [B,H,N,32]
    out    = concat heads on feature dim                # [B,N,256]

Sharding: 8 cores = 4 batches x 2 head-groups (4 heads each). Fully
data-parallel, no collectives. Each core computes out[b, :, hg*128:(hg+1)*128]
transposed ([128, 2048]); the host transposes back and concatenates.

Device-side layout is fully transposed ("T" = [feature/m, n]):
    xtT  [128(4h x 32o), 2048n]   stacked per-head xt^T (bias included)
    xCT  [128(4h x 32p), 2048n]   stacked per-head xC^T
    sT   [128m, n]     = scores[n, m]   (psum, per m-chunk per head)
    z    = sT * adjT   (adjT host-pretransposed so it is [m, n])
    alphaT = tanh(z)
    outT [128(4h x 32o), 2048n] accumulated in psum over 16 m-chunks

Engine budget per core: the kernel sits at the DVE+ACT joint capacity
floor (TRN2 matmul psum output is fp32-only, so every score element must
exit PSUM through a 1x fp32 DVE/ACT op; tanh is ACT-only at 0.83ns/col).
The LP optimum routes ~12-14 of the 128 multiply units through an ACT
cast + GPSIMD multiply and fuses the rest with the psum exit on DVE:
ACT ~= DVE ~= 135us busy. Everything else is scheduling:
 - per-mc z/alpha tiles [P,4,1024] with the tanh pipelined ONE m-chunk
   behind the multiplies (tanh(mc-1) emitted after mc's scores+mults),
   so the GPSIMD cast fires a full tanh+2 mults (~6us) before its
   consumer and ACT always has a banked z buffer to chew on.
 - no zero-seed matmuls: has_written clears are per-partition-slice on
   this HW (the baseline's diagonal xCT matmuls with start=True already
   relied on that), so heads/proj groups use start=(first k-chunk).
 - prologue: xT chunk0 DMA issued FIRST (it heads the critical chain),
   then W, bias; xT1-3 + ident on the gpsimd SWDGE queue. Only proj/xC
   chunks 0,1 gate the start; chunks 2,3 + xt4 transpose groups are
   spread one-per-mc through nh0's early m-chunks.
 - tail: mc15's tanh+heads+output copy are emitted per-q so the last
   512-col output DMA chases the last matmul by <1us.

Key facts baked into this design (from HW traces + the CoreSim cost
model source):
 - DVE tensor_tensor 2x mode needs ALL operands 2-byte; fp32 psum scores
   therefore pin the mask-multiply at 1x. TRN2 matmul cannot write bf16
   psum (is_transpose=True really is a different PE datapath - measured
   garbage), and DMA has no PSUM port, so there is no cheaper exit.
 - GPSIMD cannot access PSUM; its multiplies need an ACT cast first
   (1.0us/unit ACT buys 1.15us/unit off DVE - profitable only while
   ACT's tanh load leaves it slack).
 - K=32 scores matmuls are packed pairwise into PE row-groups
   (tile_position) with [128,2,512] psum tiles; outT uses 4-way
   col-group packing. PSUM: 3x2-bank scores slots + 2x1-bank output
   accumulators = 8 banks.
"""

import sys
import types

import numpy as np
import ml_dtypes

BF16_NP = ml_dtypes.bfloat16


def _ensure_axon_ntff_hook():
    """Provide antenv.axon_hooks if the image lacks it, so
    run_bass_kernel_spmd(trace=True) can capture NTFF profiles instead of
    crashing on the import. No-op when the real module exists."""
    try:
        import antenv.axon_hooks  # noqa: F401

        return
    except ImportError:
        pass
    mod = types.ModuleType("antenv.axon_hooks")
    _state = {"hook": None}
    mod.set_axon_ntff_profile_hook = lambda h: _state.__setitem__("hook", h)
    mod.get_axon_ntff_profile_hook = lambda: _state["hook"]
    sys.modules["antenv.axon_hooks"] = mod
    try:
        import antenv

        antenv.axon_hooks = mod
    except ImportError:
        pass
    try:
        from trn_agent_boot.trn_boot import _ntff_profile_via_ctypes

        mod.set_axon_ntff_profile_hook(
            _ntff_profile_via_ctypes("/opt/axon/libaxon_pjrt.so")
        )
    except Exception:
        pass


_ensure_axon_ntff_hook()

from concourse import bacc, mybir, tile
import concourse.bass as bass
from concourse.bass_utils import run_bass_kernel_spmd

F32 = mybir.dt.float32
BF16 = mybir.dt.bfloat16
AF = mybir.ActivationFunctionType
ALU = mybir.AluOpType

P = 128
B, N, IN_DIM, H, ATN = 4, 2048, 256, 8, 32
NH = 4                # heads per core
NCORES = 8
MC = N // P           # 16 m-chunks
IC = IN_DIM // P      # 2 contraction chunks for the input projection

# m-chunks whose (hp=0, q=0) multiply slice is routed ACT-cast -> GPSIMD
# (per n-half). The cast is emitted before tanh(mc-1), giving the Pool
# multiply ~6us of latency margin before tanh(mc) needs its z slice.
POOL_MCS = (2, 4, 6, 8, 10, 12, 14)

_CACHE = {}


def build_graph():
    nc = bacc.Bacc("TRN2", target_bir_lowering=False, debug=False)

    xT_d = nc.dram_tensor("xT", [IN_DIM, N], BF16, kind="ExternalInput")
    id_d = nc.dram_tensor("ident", [P, P], BF16, kind="ExternalInput")
    adjT_d = nc.dram_tensor("adjT", [N, N], BF16, kind="ExternalInput")
    # weights: [P, IC*NH*ATN] W-part ++ [P, ATN] C^T-part, one fast DMA
    W_d = nc.dram_tensor("Wt", [P, IC * NH * ATN + ATN], BF16, kind="ExternalInput")
    b_d = nc.dram_tensor("bias", [P, 1], F32, kind="ExternalInput")
    out_d = nc.dram_tensor("out", [P, N], BF16, kind="ExternalOutput")

    with tile.TileContext(nc) as tc:
        with (
            tc.tile_pool(name="const", bufs=1) as cp,
            tc.tile_pool(name="adj", bufs=6) as adjp,
            tc.tile_pool(name="z", bufs=5) as zp,
            tc.tile_pool(name="alpha", bufs=4) as alp,
            tc.tile_pool(name="cast", bufs=3) as scp,
            tc.tile_pool(name="ps_o", bufs=2, space="PSUM") as ps_o,
            tc.tile_pool(name="ps_s", bufs=3, space="PSUM") as ps_s,
        ):
            # xT chunk0 heads the serial critical chain (proj -> xC ->
            # scores -> mult -> tanh): issue it FIRST on the sync queue.
            # DMA issues serialize at ~0.65us each per sequencer.
            xT_sb = cp.tile([P, IC, N], BF16)
            xT_src = xT_d[:].rearrange("(c p) n -> p c n", p=P)
            nc.sync.dma_start(
                xT_sb[:, :, bass.ts(0, 512)], xT_src[:, :, bass.ts(0, 512)]
            )
            Wall_sb = cp.tile([P, IC * NH * ATN + ATN], BF16)
            nc.sync.dma_start(Wall_sb[:], W_d[:])
            b_sb = cp.tile([P, 1], F32)
            nc.sync.dma_start(b_sb[:], b_d[:])
            for nq in range(1, N // 512):
                nc.gpsimd.dma_start(
                    xT_sb[:, :, bass.ts(nq, 512)],
                    xT_src[:, :, bass.ts(nq, 512)],
                )
            ident = cp.tile([P, P], BF16)
            nc.gpsimd.dma_start(ident[:], id_d[:])
            W_sb = Wall_sb[:, : IC * NH * ATN].rearrange(
                "p (c h o) -> p c h o", c=IC, h=NH
            )
            CT_sb = Wall_sb[:, IC * NH * ATN :]

            xtT = cp.tile([P, N], BF16)
            xCT = cp.tile([P, N], BF16)
            xt4 = cp.tile([P, MC, P], BF16)
            out_sb = cp.tile([P, N], BF16)

            # --- prologue pieces ---
            def emit_xtT(nq):
                # xtT[32h+o, n] = sum_i W[h,i,o] x[n,i] + b[h,o].
                # c-outer/h-inner so the 4 col-groups run concurrently in
                # the PE array; start=True on the first c-chunk per group
                # (has_written clear is per-partition-slice). The bias
                # rides on the ACT copy out of psum.
                pt = ps_s.tile([P, 1024], F32, tag="s", name=f"pj_{nq}")
                for c in range(IC):
                    for h in range(NH):
                        nc.tensor.matmul(
                            pt[bass.ts(h, ATN), :512],
                            W_sb[:, c, h, :],
                            xT_sb[:, c, bass.ts(nq, 512)],
                            start=(c == 0),
                            stop=(c == IC - 1),
                            tile_position=(0, h * ATN),
                            skip_group_check=True,
                        )
                nc.scalar.activation(
                    xtT[:, bass.ts(nq, 512)], pt[:, :512], AF.Identity, bias=b_sb[:]
                )

            def emit_xCT(nq, on_act=True):
                # xCT[32h+p, n] = sum_o C[h,p,o] xt[n,o]; diagonal 32x32
                # tiles run concurrently in distinct row+col groups.
                pt = ps_s.tile([P, 1024], F32, tag="s", name=f"xc_{nq}")
                for h in range(NH):
                    nc.tensor.matmul(
                        pt[bass.ts(h, ATN), :512],
                        CT_sb[bass.ts(h, ATN), :],
                        xtT[bass.ts(h, ATN), bass.ts(nq, 512)],
                        start=True,
                        stop=True,
                        tile_position=(h * ATN, h * ATN),
                        skip_group_check=True,
                    )
                if on_act:
                    nc.scalar.copy(xCT[:, bass.ts(nq, 512)], pt[:, :512])
                else:
                    nc.vector.tensor_copy(xCT[:, bass.ts(nq, 512)], pt[:, :512])

            def emit_xt4(g):
                # xt4[m_local, mc, f] = xt[mc*128+m_local, f]: PE transposes
                # of xtT, 4 m-chunks per psum tile (cycled through a ps_s
                # slot). Copies on DVE (bf16 psum -> bf16 sbuf, 2x path).
                pt = ps_s.tile([P, 4, P], BF16, tag="s", name=f"tr_{g}")
                for k in range(4):
                    nc.tensor.transpose(
                        pt[:, k, :], xtT[:, bass.ts(4 * g + k, P)], ident[:]
                    )
                nc.vector.tensor_copy(xt4[:, bass.ds(4 * g, 4), :], pt[:])

            # Only projection chunks 0,1 (and transpose group 0) gate the
            # first m-chunk; chunks 2,3 and the later transpose groups are
            # emitted inside the main loop where PE has slack.
            for nq in range(2):
                emit_xtT(nq)
                emit_xCT(nq)
            emit_xt4(0)
            PROLOG_AT = {1: lambda: emit_xtT(2),
                         2: lambda: emit_xCT(2, on_act=False),
                         3: lambda: emit_xt4(1),
                         4: lambda: emit_xtT(3),
                         5: lambda: emit_xCT(3, on_act=False),
                         6: lambda: emit_xt4(2),
                         7: lambda: emit_xt4(3)}

            # --- main loop: n-half outer, m-chunks inner, tanh lagging the
            # multiplies by one m-chunk ---
            NHALF = N // 1024
            for nh in range(NHALF):
                # Two 1-bank accumulators; the q0 output copy fires as soon
                # as q0's last matmul stops, overlapping q1's.
                po_q = [
                    ps_o.tile([P, 512], F32, tag="po", name=f"po_{nh}_{q}")
                    for q in range(2)
                ]

                def emit_heads(mc, alpha, q, nh=nh):
                    for h in range(NH):
                        nc.tensor.matmul(
                            po_q[q][bass.ts(h, ATN), :],
                            xt4[:, mc, bass.ts(h, ATN)],
                            alpha[:, h, bass.ts(q, 512)],
                            start=(mc == 0),
                            stop=(mc == MC - 1),
                            tile_position=(0, h * ATN),
                            skip_group_check=True,
                        )

                zbuf = [None, None]
                abuf = [None, None]

                def emit_tanh(mc, nh=nh):
                    alpha = alp.tile([P, NH, 1024], BF16, tag="alpha",
                                     name=f"al_{nh}_{mc}")
                    abuf[mc % 2] = alpha
                    nc.scalar.activation(alpha[:], zbuf[mc % 2][:], AF.Tanh)

                for mc in range(MC):
                    adjt = adjp.tile([P, 1024], BF16, tag="adj")
                    nc.sync.dma_start(
                        adjt[:], adjT_d[bass.ts(mc, P), bass.ds(nh * 1024, 1024)]
                    )
                    zb = zp.tile([P, NH, 1024], BF16, tag="z",
                                 name=f"zb_{nh}_{mc}")
                    zbuf[mc % 2] = zb
                    pool_mc = mc in POOL_MCS
                    # heads(mc-2) BEFORE this mc's scores: its tanh landed a
                    # full m-chunk ago, so PE never parks waiting instrs in
                    # front of the scores stream (4-deep wait queue would
                    # head-of-line block DVE's multiply feed otherwise).
                    if mc >= 2:
                        for q in range(2):
                            emit_heads(mc - 2, abuf[mc % 2], q)
                    for q in range(2):
                        for hp in range(NH // 2):
                            s2 = ps_s.tile([P, 2, 512], F32, tag="s")
                            for j in range(2):
                                h = 2 * hp + j
                                nc.tensor.matmul(
                                    s2[:, j, :],
                                    xtT[bass.ts(h, ATN), bass.ts(mc, P)],
                                    xCT[
                                        bass.ts(h, ATN),
                                        bass.ds(nh * 1024 + q * 512, 512),
                                    ],
                                    start=True,
                                    stop=True,
                                    tile_position=(h * ATN, 0),
                                    skip_group_check=True,
                                )
                            zsl = zb[:, bass.ds(2 * hp, 2), bass.ts(q, 512)]
                            adj_b = adjt[:, None, bass.ts(q, 512)].to_broadcast(
                                (P, 2, 512)
                            )
                            if pool_mc and q == 0 and hp == 0:
                                # ACT casts psum->bf16; the otherwise-idle
                                # GPSIMD engine does the multiply (it has
                                # no PSUM port, hence the cast).
                                sc = scp.tile([P, 2, 512], BF16, tag="cast")
                                nc.scalar.copy(sc[:], s2[:])
                                nc.gpsimd.tensor_tensor(zsl, sc[:], adj_b, ALU.mult)
                            else:
                                nc.vector.tensor_tensor(zsl, s2[:], adj_b, ALU.mult)
                    if nh == 0 and mc in PROLOG_AT:
                        PROLOG_AT[mc]()
                    if mc >= 1:
                        emit_tanh(mc - 1)
                # tail: heads(14), then per-q tanh(15) + heads(15) + output
                # copy + DMA so the last 512-col DMA chases the last matmul
                for q in range(2):
                    emit_heads(MC - 2, abuf[(MC - 2) % 2], q)
                alpha = alp.tile([P, NH, 1024], BF16, tag="alpha",
                                 name=f"al_{nh}_{MC - 1}")
                for q in range(2):
                    nc.scalar.activation(
                        alpha[:, :, bass.ts(q, 512)],
                        zbuf[(MC - 1) % 2][:, :, bass.ts(q, 512)],
                        AF.Tanh,
                    )
                    emit_heads(MC - 1, alpha, q)
                    nc.vector.tensor_copy(
                        out_sb[:, bass.ds(nh * 1024 + q * 512, 512)],
                        po_q[q][:],
                    )
                    nc.sync.dma_start(
                        out_d[:, bass.ds(nh * 1024 + q * 512, 512)],
                        out_sb[:, bass.ds(nh * 1024 + q * 512, 512)],
                    )

    nc.compile()
    return nc


def _get_graph():
    if "nc" not in _CACHE:
        _CACHE["nc"] = build_graph()
    return _CACHE["nc"]


def make_in_maps(x, adj, W, b, C):
    in_maps = []
    for core in range(NCORES):
        bb = core // 2
        hg = core % 2
        hs = slice(hg * NH, (hg + 1) * NH)
        Wt = (
            W[hs]
            .reshape(NH, IC, P, ATN)
            .transpose(2, 1, 0, 3)
            .reshape(P, IC * NH * ATN)
        )
        CTt = C[hs].transpose(0, 2, 1).reshape(NH * ATN, ATN)
        in_maps.append(
            {
                "xT": np.ascontiguousarray(x[bb].T).astype(BF16_NP),
                "ident": np.eye(P, dtype=np.float32).astype(BF16_NP),
                "adjT": np.ascontiguousarray(adj[bb].T).astype(BF16_NP),
                "Wt": np.ascontiguousarray(
                    np.concatenate([Wt, CTt], axis=1)
                ).astype(BF16_NP),
                "bias": np.ascontiguousarray(b[hs].reshape(P, 1)),
            }
        )
    return in_maps


LAST_RESULT = None


def kernel(x, adj, W, b, C):
    global LAST_RESULT
    x = np.asarray(x, dtype=np.float32)
    adj = np.asarray(adj, dtype=np.float32)
    W = np.asarray(W, dtype=np.float32)
    b = np.asarray(b, dtype=np.float32)
    C = np.asarray(C, dtype=np.float32)

    nc = _get_graph()
    in_maps = make_in_maps(x, adj, W, b, C)
    res = run_bass_kernel_spmd(nc, in_maps, core_ids=list(range(NCORES)))
    LAST_RESULT = res

    out = np.empty((B, N, H * ATN), dtype=np.float32)
    for core in range(NCORES):
        bb = core // 2
        hg = core % 2
        out[bb, :, hg * P : (hg + 1) * P] = (
            res.results[core]["out"].astype(np.float32).T
        )
    return out


# revision 7
# speedup vs baseline: 1.2181x; 1.0230x over previous
"""Trainium2 Bass kernel for per-head bilinear graph attention.

Reference computation (B=4, N=2048, IN=256, H=8, ATN=32):
    xt     = einsum('bni,hio->bhno', x, W) + b          # [B,H,N,32]
    xC     = einsum('bhno,hpo->bhnp', xt, C)            # [B,H,N,32]
    scores = einsum('bhnp,bhmp->bhnm', xC, xt)          # [B,H,N,N]
    alpha  = tanh(scores * adj[:,None])                 # [B,H,N,N]
    heads  = einsum('bhnm,bhmo->bhno', alpha, xt)       # [B,H,N,32]
    out    = concat heads on feature dim                # [B,N,256]

Sharding: 8 cores = 4 batches x 2 head-groups (4 heads each). Fully
data-parallel, no collectives. Each core computes out[b, :, hg*128:(hg+1)*128]
transposed ([128, 2048]); the host transposes back and concatenates.

Device-side layout is fully transposed ("T" = [feature/m, n]):
    xtT  [128(4h x 32o), 2048n]   stacked per-head xt^T (bias included)
    xCT  [128(4h x 32p), 2048n]   stacked per-head xC^T
    sT   [128m, n]     = scores[n, m]   (psum, per m-chunk per head)
    z    = sT * adjT   (adjT host-pretransposed so it is [m, n])
    alphaT = tanh(z)
    outT [128(4h x 32o), 2048n] accumulated in psum over 16 m-chunks

Engine budget per core: the kernel sits at the DVE+ACT joint capacity
floor (TRN2 matmul psum output is fp32-only, so every score element must
exit PSUM through a 1x fp32 DVE/ACT op; tanh is ACT-only at 0.83ns/col).
The LP optimum routes ~12-14 of the 128 multiply units through an ACT
cast + GPSIMD multiply and fuses the rest with the psum exit on DVE:
ACT ~= DVE ~= 135us busy. Everything else is scheduling:
 - per-mc z/alpha tiles [P,4,1024] with the tanh pipelined ONE m-chunk
   behind the multiplies (tanh(mc-1) emitted after mc's scores+mults),
   so the GPSIMD cast fires a full tanh+2 mults (~6us) before its
   consumer and ACT always has a banked z buffer to chew on.
 - no zero-seed matmuls: has_written clears are per-partition-slice on
   this HW (the baseline's diagonal xCT matmuls with start=True already
   relied on that), so heads/proj groups use start=(first k-chunk).
 - prologue: xT chunk0 DMA issued FIRST (it heads the critical chain),
   then W, bias; xT1-3 + ident on the gpsimd SWDGE queue. Only proj/xC
   chunks 0,1 gate the start; chunks 2,3 + xt4 transpose groups are
   spread one-per-mc through nh0's early m-chunks.
 - tail: mc15's tanh+heads+output copy are emitted per-q so the last
   512-col output DMA chases the last matmul by <1us.

Key facts baked into this design (from HW traces + the CoreSim cost
model source):
 - DVE tensor_tensor 2x mode needs ALL operands 2-byte; fp32 psum scores
   therefore pin the mask-multiply at 1x. TRN2 matmul cannot write bf16
   psum (is_transpose=True really is a different PE datapath - measured
   garbage), and DMA has no PSUM port, so there is no cheaper exit.
 - GPSIMD cannot access PSUM; its multiplies need an ACT cast first
   (1.0us/unit ACT buys 1.15us/unit off DVE - profitable only while
   ACT's tanh load leaves it slack).
 - K=32 scores matmuls are packed pairwise into PE row-groups
   (tile_position) with [128,2,512] psum tiles; outT uses 4-way
   col-group packing. PSUM: 3x2-bank scores slots + 2x1-bank output
   accumulators = 8 banks.
"""

import sys
import types

import numpy as np
import ml_dtypes

BF16_NP = ml_dtypes.bfloat16


def _ensure_axon_ntff_hook():
    """Provide antenv.axon_hooks if the image lacks it, so
    run_bass_kernel_spmd(trace=True) can capture NTFF profiles instead of
    crashing on the import. No-op when the real module exists."""
    try:
        import antenv.axon_hooks  # noqa: F401

        return
    except ImportError:
        pass
    mod = types.ModuleType("antenv.axon_hooks")
    _state = {"hook": None}
    mod.set_axon_ntff_profile_hook = lambda h: _state.__setitem__("hook", h)
    mod.get_axon_ntff_profile_hook = lambda: _state["hook"]
    sys.modules["antenv.axon_hooks"] = mod
    try:
        import antenv

        antenv.axon_hooks = mod
    except ImportError:
        pass
    try:
        from trn_agent_boot.trn_boot import _ntff_profile_via_ctypes

        mod.set_axon_ntff_profile_hook(
            _ntff_profile_via_ctypes("/opt/axon/libaxon_pjrt.so")
        )
    except Exception:
        pass


_ensure_axon_ntff_hook()

from concourse import bacc, mybir, tile
import concourse.bass as bass
from concourse.bass_utils import run_bass_kernel_spmd

F32 = mybir.dt.float32
BF16 = mybir.dt.bfloat16
AF = mybir.ActivationFunctionType
ALU = mybir.AluOpType

P = 128
B, N, IN_DIM, H, ATN = 4, 2048, 256, 8, 32
NH = 4                # heads per core
NCORES = 8
MC = N // P           # 16 m-chunks
IC = IN_DIM // P      # 2 contraction chunks for the input projection

# m-chunks whose (hp=0, q=0) multiply slice is routed ACT-cast -> GPSIMD
# (per n-half). The cast is emitted before tanh(mc-1), giving the Pool
# multiply ~6us of latency margin before tanh(mc) needs its z slice.
POOL_MCS = (2, 4, 6, 8, 10, 12, 14)

_CACHE = {}


def build_graph():
    nc = bacc.Bacc("TRN2", target_bir_lowering=False, debug=False)

    xT_d = nc.dram_tensor("xT", [IN_DIM, N], BF16, kind="ExternalInput")
    id_d = nc.dram_tensor("ident", [P, P], BF16, kind="ExternalInput")
    adjT_d = nc.dram_tensor("adjT", [N, N], BF16, kind="ExternalInput")
    # weights: [P, IC*NH*ATN] W-part ++ [P, ATN] C^T-part, one fast DMA
    W_d = nc.dram_tensor("Wt", [P, IC * NH * ATN + ATN], BF16, kind="ExternalInput")
    b_d = nc.dram_tensor("bias", [P, 1], F32, kind="ExternalInput")
    out_d = nc.dram_tensor("out", [P, N], BF16, kind="ExternalOutput")

    with tile.TileContext(nc) as tc:
        with (
            tc.tile_pool(name="const", bufs=1) as cp,
            tc.tile_pool(name="adj", bufs=6) as adjp,
            tc.tile_pool(name="z", bufs=5) as zp,
            tc.tile_pool(name="alpha", bufs=4) as alp,
            tc.tile_pool(name="cast", bufs=3) as scp,
            tc.tile_pool(name="ps_o", bufs=2, space="PSUM") as ps_o,
            tc.tile_pool(name="ps_s", bufs=3, space="PSUM") as ps_s,
        ):
            # xT chunk0 heads the serial critical chain (proj -> xC ->
            # scores -> mult -> tanh): issue it FIRST on the sync queue.
            # DMA issues serialize at ~0.65us each per sequencer.
            xT_sb = cp.tile([P, IC, N], BF16)
            xT_src = xT_d[:].rearrange("(c p) n -> p c n", p=P)
            nc.sync.dma_start(
                xT_sb[:, :, bass.ts(0, 512)], xT_src[:, :, bass.ts(0, 512)]
            )
            Wall_sb = cp.tile([P, IC * NH * ATN + ATN], BF16)
            nc.sync.dma_start(Wall_sb[:], W_d[:])
            b_sb = cp.tile([P, 1], F32)
            nc.sync.dma_start(b_sb[:], b_d[:])
            for nq in range(1, N // 512):
                nc.gpsimd.dma_start(
                    xT_sb[:, :, bass.ts(nq, 512)],
                    xT_src[:, :, bass.ts(nq, 512)],
                )
            ident = cp.tile([P, P], BF16)
            nc.gpsimd.dma_start(ident[:], id_d[:])
            W_sb = Wall_sb[:, : IC * NH * ATN].rearrange(
                "p (c h o) -> p c h o", c=IC, h=NH
            )
            CT_sb = Wall_sb[:, IC * NH * ATN :]

            xtT = cp.tile([P, N], BF16)
            xCT = cp.tile([P, N], BF16)
            xt4 = cp.tile([P, MC, P], BF16)
            out_sb = cp.tile([P, N], BF16)

            # --- prologue pieces ---
            def emit_xtT(nq):
                # xtT[32h+o, n] = sum_i W[h,i,o] x[n,i] + b[h,o].
                # c-outer/h-inner so the 4 col-groups run concurrently in
                # the PE array; start=True on the first c-chunk per group
                # (has_written clear is per-partition-slice). The bias
                # rides on the ACT copy out of psum.
                pt = ps_s.tile([P, 1024], F32, tag="s", name=f"pj_{nq}")
                for c in range(IC):
                    for h in range(NH):
                        nc.tensor.matmul(
                            pt[bass.ts(h, ATN), :512],
                            W_sb[:, c, h, :],
                            xT_sb[:, c, bass.ts(nq, 512)],
                            start=(c == 0),
                            stop=(c == IC - 1),
                            tile_position=(0, h * ATN),
                            skip_group_check=True,
                        )
                nc.scalar.activation(
                    xtT[:, bass.ts(nq, 512)], pt[:, :512], AF.Identity, bias=b_sb[:]
                )

            def emit_xCT(nq, on_act=True):
                # xCT[32h+p, n] = sum_o C[h,p,o] xt[n,o]; diagonal 32x32
                # tiles run concurrently in distinct row+col groups.
                pt = ps_s.tile([P, 1024], F32, tag="s", name=f"xc_{nq}")
                for h in range(NH):
                    nc.tensor.matmul(
                        pt[bass.ts(h, ATN), :512],
                        CT_sb[bass.ts(h, ATN), :],
                        xtT[bass.ts(h, ATN), bass.ts(nq, 512)],
                        start=True,
                        stop=True,
                        tile_position=(h * ATN, h * ATN),
                        skip_group_check=True,
                    )
                if on_act:
                    nc.scalar.copy(xCT[:, bass.ts(nq, 512)], pt[:, :512])
                else:
                    nc.vector.tensor_copy(xCT[:, bass.ts(nq, 512)], pt[:, :512])

            def emit_xt4(g):
                # xt4[m_local, mc, f] = xt[mc*128+m_local, f]: PE transposes
                # of xtT, 4 m-chunks per psum tile (cycled through a ps_s
                # slot). Copies on DVE (bf16 psum -> bf16 sbuf, 2x path).
                pt = ps_s.tile([P, 4, P], BF16, tag="s", name=f"tr_{g}")
                for k in range(4):
                    nc.tensor.transpose(
                        pt[:, k, :], xtT[:, bass.ts(4 * g + k, P)], ident[:]
                    )
                nc.vector.tensor_copy(xt4[:, bass.ds(4 * g, 4), :], pt[:])

            # Only projection chunks 0,1 (and transpose group 0) gate the
            # first m-chunk; chunks 2,3 and the later transpose groups are
            # emitted inside the main loop where PE has slack.
            for nq in range(2):
                emit_xtT(nq)
                emit_xCT(nq)
            emit_xt4(0)
            PROLOG_AT = {1: lambda: emit_xtT(2),
                         2: lambda: emit_xCT(2, on_act=False),
                         3: lambda: emit_xt4(1),
                         4: lambda: emit_xtT(3),
                         5: lambda: emit_xCT(3, on_act=False),
                         6: lambda: emit_xt4(2),
                         7: lambda: emit_xt4(3)}

            # --- main loop: n-half outer, m-chunks inner, tanh lagging the
            # multiplies by one m-chunk ---
            NHALF = N // 1024
            for nh in range(NHALF):
                # Two 1-bank accumulators; the q0 output copy fires as soon
                # as q0's last matmul stops, overlapping q1's.
                po_q = [
                    ps_o.tile([P, 512], F32, tag="po", name=f"po_{nh}_{q}")
                    for q in range(2)
                ]

                def emit_heads(mc, alpha, q, nh=nh):
                    for h in range(NH):
                        nc.tensor.matmul(
                            po_q[q][bass.ts(h, ATN), :],
                            xt4[:, mc, bass.ts(h, ATN)],
                            alpha[:, h, bass.ts(q, 512)],
                            start=(mc == 0),
                            stop=(mc == MC - 1),
                            tile_position=(0, h * ATN),
                            skip_group_check=True,
                        )

                zbuf = [None, None]
                abuf = [None, None]

                def emit_tanh(mc, nh=nh):
                    alpha = alp.tile([P, NH, 1024], BF16, tag="alpha",
                                     name=f"al_{nh}_{mc}")
                    abuf[mc % 2] = alpha
                    nc.scalar.activation(alpha[:], zbuf[mc % 2][:], AF.Tanh)

                for mc in range(MC):
                    adjt = adjp.tile([P, 1024], BF16, tag="adj")
                    nc.sync.dma_start(
                        adjt[:], adjT_d[bass.ts(mc, P), bass.ds(nh * 1024, 1024)]
                    )
                    zb = zp.tile([P, NH, 1024], BF16, tag="z",
                                 name=f"zb_{nh}_{mc}")
                    zbuf[mc % 2] = zb
                    pool_mc = mc in POOL_MCS
                    # heads(mc-2) interleaved BETWEEN scores groups: its tanh
                    # landed a full m-chunk ago so it never parks waiting
                    # instrs in front of the scores stream (the 4-deep PE
                    # wait queue would head-of-line block DVE's multiply
                    # feed), and splitting the 8-matmul heads burst in two
                    # keeps the 3-slot scores window from draining.
                    for q in range(2):
                        if mc >= 2:
                            emit_heads(mc - 2, abuf[mc % 2], q)
                        for hp in range(NH // 2):
                            s2 = ps_s.tile([P, 2, 512], F32, tag="s")
                            for j in range(2):
                                h = 2 * hp + j
                                nc.tensor.matmul(
                                    s2[:, j, :],
                                    xtT[bass.ts(h, ATN), bass.ts(mc, P)],
                                    xCT[
                                        bass.ts(h, ATN),
                                        bass.ds(nh * 1024 + q * 512, 512),
                                    ],
                                    start=True,
                                    stop=True,
                                    tile_position=(h * ATN, 0),
                                    skip_group_check=True,
                                )
                            zsl = zb[:, bass.ds(2 * hp, 2), bass.ts(q, 512)]
                            adj_b = adjt[:, None, bass.ts(q, 512)].to_broadcast(
                                (P, 2, 512)
                            )
                            if pool_mc and q == 0 and hp == 0:
                                # ACT casts psum->bf16; the otherwise-idle
                                # GPSIMD engine does the multiply (it has
                                # no PSUM port, hence the cast).
                                sc = scp.tile([P, 2, 512], BF16, tag="cast")
                                nc.scalar.copy(sc[:], s2[:])
                                nc.gpsimd.tensor_tensor(zsl, sc[:], adj_b, ALU.mult)
                            else:
                                nc.vector.tensor_tensor(zsl, s2[:], adj_b, ALU.mult)
                    if nh == 0 and mc in PROLOG_AT:
                        PROLOG_AT[mc]()
                    if mc >= 1:
                        emit_tanh(mc - 1)
                # tail: heads(14), then per-q tanh(15) + heads(15) + output
                # copy + DMA so the last 512-col DMA chases the last matmul
                for q in range(2):
                    emit_heads(MC - 2, abuf[(MC - 2) % 2], q)
                alpha = alp.tile([P, NH, 1024], BF16, tag="alpha",
                                 name=f"al_{nh}_{MC - 1}")
                for q in range(2):
                    nc.scalar.activation(
                        alpha[:, :, bass.ts(q, 512)],
                        zbuf[(MC - 1) % 2][:, :, bass.ts(q, 512)],
                        AF.Tanh,
                    )
                    emit_heads(MC - 1, alpha, q)
                    nc.vector.tensor_copy(
                        out_sb[:, bass.ds(nh * 1024 + q * 512, 512)],
                        po_q[q][:],
                    )
                    nc.sync.dma_start(
                        out_d[:, bass.ds(nh * 1024 + q * 512, 512)],
                        out_sb[:, bass.ds(nh * 1024 + q * 512, 512)],
                    )

    nc.compile()
    return nc


def _get_graph():
    if "nc" not in _CACHE:
        _CACHE["nc"] = build_graph()
    return _CACHE["nc"]


def make_in_maps(x, adj, W, b, C):
    in_maps = []
    for core in range(NCORES):
        bb = core // 2
        hg = core % 2
        hs = slice(hg * NH, (hg + 1) * NH)
        Wt = (
            W[hs]
            .reshape(NH, IC, P, ATN)
            .transpose(2, 1, 0, 3)
            .reshape(P, IC * NH * ATN)
        )
        CTt = C[hs].transpose(0, 2, 1).reshape(NH * ATN, ATN)
        in_maps.append(
            {
                "xT": np.ascontiguousarray(x[bb].T).astype(BF16_NP),
                "ident": np.eye(P, dtype=np.float32).astype(BF16_NP),
                "adjT": np.ascontiguousarray(adj[bb].T).astype(BF16_NP),
                "Wt": np.ascontiguousarray(
                    np.concatenate([Wt, CTt], axis=1)
                ).astype(BF16_NP),
                "bias": np.ascontiguousarray(b[hs].reshape(P, 1)),
            }
        )
    return in_maps


LAST_RESULT = None


def kernel(x, adj, W, b, C):
    global LAST_RESULT
    x = np.asarray(x, dtype=np.float32)
    adj = np.asarray(adj, dtype=np.float32)
    W = np.asarray(W, dtype=np.float32)
    b = np.asarray(b, dtype=np.float32)
    C = np.asarray(C, dtype=np.float32)

    nc = _get_graph()
    in_maps = make_in_maps(x, adj, W, b, C)
    res = run_bass_kernel_spmd(nc, in_maps, core_ids=list(range(NCORES)))
    LAST_RESULT = res

    out = np.empty((B, N, H * ATN), dtype=np.float32)
    for core in range(NCORES):
        bb = core // 2
        hg = core % 2
        out[bb, :, hg * P : (hg + 1) * P] = (
            res.results[core]["out"].astype(np.float32).T
        )
    return out
